# revision 58
# baseline (speedup 1.0000x reference)
"""Causal single-head attention on 8 Trainium2 NeuronCores.

Problem: x[4, 4096, 1024], Wq/Wk/Wv[1024, 64] ->
  out = softmax(causal(Q K^T / 8)) V   per batch, fp32.

Sharding: core i handles batch b = i//2 with query-chunk parity p = i%2
(512-wide query chunks; core p owns global chunks {p, 2+p, 4+p, 6+p}).
Both cores of a pair project the full x[b] (pair collectives measured too
slow here, ~40us each, to be worth deduplicating the projections).

The SPMD program is identical on all cores. Parity enters only through data:
  - the host-sliced x columns (own 4 chunks, then partner 4 chunks)
  - multiplicative 0/1 mask planes [128, o, d, 1024]: (o=own, u=j) is the
    diagonal staircase (parity-independent values); (o=partner, u=j) is
    all-zero on p=0 (pad work for SPMD balance) and all-ones on p=1.
K/V state is indexed [own/partner][u]; attention visits key chunks in
per-core order, which is legal because softmax and PV accumulation are
order-free over keys.

Pipeline structure: each score pair computes two 128-key tiles CONCURRENTLY
(dual-tile 64x128 row-tiled PE: T0 on partitions 0-63, T8 on 64-127) into one
2-bank PSUM tile [128, 1024]; exp is one scalar ACTIVATE per pair; the causal
mask is a multiplicative 0/1 bf16 vector op AFTER exp, off the scores->exp
chain; PV matmuls lag scores by LAG pairs so the in-order PE queue never
waits on the scalar engine; projection matmuls for later chunks interleave
into the attention loop as PE filler (keeps HAM at K=8/8 = 2.4 GHz).

On-device layout: scores are computed transposed (S^T[k, q] = K^T.T Q^T per
128x512 block). PV then uses the softmax'd P^T tile as the *stationary*
operand and V (augmented with a ones column accumulating the softmax
denominator) as the moving operand: out[q, h+1] = P^T.T @ V_aug lands
directly in natural [q, h] layout, so no final PE transpose is needed and
normalization reads the PSUM accumulator in place.

Operands are bf16 (enables Fast Weight Load on 128-col stationaries and
halves DMA); accumulation is fp32 in PSUM. Measured rel err ~4.4e-3.
"""

import numpy as np

import concourse.bacc as bacc
import concourse.mybir as mybir
import concourse.tile as tile
from concourse.bass_utils import run_bass_kernel_spmd

# Problem dims
B, T, C, HS = 4, 4096, 1024, 64
P = 128           # partitions
CH = 512          # query-chunk width
NCH = T // CH     # 8 global chunks
NSLOT = NCH // 2  # 4 local query slots / own chunks per core
CSUB = C // P     # 8 contraction subtiles
KT_PER_CH = CH // P   # 4 key tiles (128) per chunk
QS_PER_CH = CH // P   # 4 query subtiles (128) per chunk
NEG = -1.0e9
LAG = 2           # pair-pipeline depth: PV for pair g emitted at step g+LAG

IN_DT = mybir.dt.bfloat16  # matmul operand storage dtype


def _build_program():
    nc = bacc.Bacc("TRN2")
    f32 = mybir.dt.float32
    EXP = mybir.ActivationFunctionType.Exp

    # host-pretiled x: [ci, local chunk, co, t] with own u0..u3 then partner
    # u0..u3 on the chunk axis; each chunk half-DMA reads 4KB/partition lines
    xT = nc.dram_tensor(
        "xT", [P, NCH, CSUB, CH], IN_DT, kind="ExternalInput"
    ).ap()
    wqk = nc.dram_tensor("wqk", [C, 2 * HS], IN_DT, kind="ExternalInput").ap()
    wv = nc.dram_tensor("wv", [C, HS], IN_DT, kind="ExternalInput").ap()
    # causal mask planes: [128, o, d, 1024]
    mask2_d = nc.dram_tensor(
        "mask2", [P, 2, 2, 2 * CH], IN_DT, kind="ExternalInput"
    ).ap()
    out_d = nc.dram_tensor("out", [NSLOT * CH, HS], f32, kind="ExternalOutput").ap()

    wqk_r = wqk.rearrange("(co ci) m -> ci co m", ci=P)    # [128, 8, 128]
    wv_r = wv.rearrange("(co ci) m -> ci co m", ci=P)      # [128, 8, 64]

    with tile.TileContext(nc) as tc:
        with (
            tc.tile_pool(name="const", bufs=1) as const_pool,
            tc.tile_pool(name="persist", bufs=1) as persist,
            tc.tile_pool(name="xin", bufs=8) as xpool,
            tc.tile_pool(name="pt", bufs=6) as pt_pool,
            tc.tile_pool(name="fin", bufs=2) as fin_pool,
            tc.tile_pool(name="proj_ps", bufs=2, space="PSUM") as proj_ps,
            tc.tile_pool(name="st_ps", bufs=2, space="PSUM") as st_ps,
            tc.tile_pool(name="ot_ps", bufs=2, space="PSUM") as ot_ps,
        ):
            # ---- constants (gpsimd DMA queue; sync queue is for x) ----
            wqk_sb = const_pool.tile([P, CSUB, 2 * HS], IN_DT)
            wv_sb = const_pool.tile([P, CSUB, HS], IN_DT)
            mask2_sb = const_pool.tile([P, 2, 2, 2 * CH], IN_DT)
            nc.gpsimd.dma_start(wqk_sb[:], wqk_r)

            # K^T packed for dual-tile scores: keys 0-255 of a chunk in
            # partitions 0-63 (PE tile T0), keys 256-511 in partitions
            # 64-127 (tile T8) -> two score matmuls run CONCURRENTLY in the
            # row-tiled 64x128 PE configuration.
            kt_all = persist.tile([P, 2, NSLOT, CH // 2], IN_DT)
            v_all = persist.tile([P, 2, NSLOT, KT_PER_CH, HS + 1], IN_DT)
            # Q^T duplicated into both partition halves (T0 and T8 streams)
            qt_slot = persist.tile([P, NSLOT, CH], IN_DT)
            # 0x3F80 = 1.0 in bf16
            nc.vector.memset(
                v_all[:, :, :, :, HS : HS + 1].bitcast(mybir.dt.uint16), 0x3F80
            )



            # ---- local projection of one chunk --------------------------
            def proj_chunk_steps(lc, o, u):
                xc = xpool.tile([P, CSUB, CH], IN_DT, tag="xc")
                nc.sync.dma_start(xc[:, 0:4, :], xT[:, lc, 0:4, :])
                yield
                nc.sync.dma_start(xc[:, 4:8, :], xT[:, lc, 4:8, :])
                if lc == 0:
                    # deferred consts: wv needed at ~18us, mask at ~22us --
                    # keep them off the DMA engines while xc(0) streams
                    nc.gpsimd.dma_start(wv_sb[:], wv_r)
                    nc.gpsimd.dma_start(mask2_sb[:], mask2_d)
                yield

                # Q^T (rows 0:64) and K^T (rows 64:128), stacked projection
                qk_ps = proj_ps.tile([P, CH], f32, tag="proj")
                for cs in range(CSUB):
                    nc.tensor.matmul(
                        qk_ps[:],
                        lhsT=wqk_sb[:, cs, :],
                        rhs=xc[:, cs, :],
                        start=(cs == 0),
                        stop=(cs == CSUB - 1),
                    )
                    yield
                if o == 0:
                    nc.vector.tensor_copy(qt_slot[0:HS, u, :], qk_ps[0:HS, :])
                    nc.vector.tensor_copy(qt_slot[HS:P, u, :], qk_ps[0:HS, :])
                    yield
                nc.vector.tensor_copy(
                    kt_all[0:HS, o, u, :], qk_ps[HS:P, 0 : CH // 2]
                )
                nc.vector.tensor_copy(
                    kt_all[HS:P, o, u, :], qk_ps[HS:P, CH // 2 : CH]
                )
                yield

                # V natural ([t, h]) via x^T blocks as stationary operand
                v_ps = proj_ps.tile([P, KT_PER_CH, HS], f32, tag="proj")
                for tt in range(KT_PER_CH):
                    for cs in range(CSUB):
                        nc.tensor.matmul(
                            v_ps[:, tt, :],
                            lhsT=xc[:, cs, tt * P : (tt + 1) * P],
                            rhs=wv_sb[:, cs, :],
                            start=(cs == 0),
                            stop=(cs == CSUB - 1),
                        )
                    yield
                nc.vector.tensor_copy(v_all[:, o, u, :, 0:HS], v_ps[:])
                yield

            # emission order: own chunk j carries slot j's Q^T, so own
            # chunks come first within each causal stage; the partner diag
            # chunk is only needed by the last two pairs of its slot
            plan = [
                (0, 0, 0),   # own u0
                (4, 1, 0),   # partner u0
                (1, 0, 1),   # own u1
                (5, 1, 1),   # partner u1
                (2, 0, 2),   # own u2
                (6, 1, 2),   # partner u2
                (3, 0, 3),   # own u3
                (7, 1, 3),   # partner u3
            ]

            def steps_of(spec):
                lc, o, u = spec
                return 16 + (1 if o == 0 else 0)

            def chained(gens):
                for g in gens:
                    yield from g

            projgen = chained(proj_chunk_steps(*spec) for spec in plan)
            cum = np.cumsum([steps_of(s) for s in plan]).tolist()
            # emission watermark per projected chunk (o, u)
            chunk_pos = {
                (o, u): cum[i] for i, (lc, o, u) in enumerate(plan)
            }
            pumped = [0]

            def pump(n):
                for _ in range(n):
                    if next(projgen, "done") == "done":
                        return
                    pumped[0] += 1

            pump(chunk_pos[(0, 0)])

            # ---- attention: one flat software pipeline across all slots --
            # global pair list: (j, o, u, d); within a slot the diagonal/pad
            # chunks (u == j) come last
            pairs = []
            slot_first = []
            slot_last = []
            for j in range(NSLOT):
                slot_first.append(len(pairs))
                pairs += [
                    (j, o, u, d)
                    for u in range(j + 1)
                    for o in range(2)
                    for d in range(2)
                ]
                slot_last.append(len(pairs) - 1)
            GT = len(pairs)

            # natural-layout PV accumulator: [q, qs, h+1], one PSUM bank
            ots = {}
            pts = {}

            def emit_scores(g):
                j, o, u, d = pairs[g]
                st = st_ps.tile([P, 2 * CH], f32, tag="st")
                # st half h holds key tile d + 2h: h=0 on PE tile T0
                # (partitions 0-63), h=1 on T8 (64-127), concurrently
                for h in range(2):
                    r0 = h * HS
                    nc.tensor.matmul(
                        st[:, h * CH : (h + 1) * CH],
                        lhsT=kt_all[r0 : r0 + HS, o, u, d * P : (d + 1) * P],
                        rhs=qt_slot[r0 : r0 + HS, j, :],
                        start=True,
                        stop=True,
                    )
                pt = pt_pool.tile([P, 2 * CH], IN_DT, tag="pt")
                nc.scalar.activation(pt[:], st[:], EXP, scale=float(HS) ** -0.5)
                if u == j:
                    # multiplicative 0/1 causal mask AFTER exp: off the
                    # scores->exp critical chain (PV's LAG absorbs it) and
                    # bf16 runs at 2x DVE rate vs the old f32 PSUM add
                    nc.vector.tensor_mul(pt[:], pt[:], mask2_sb[:, o, d, :])
                pts[g] = pt

            def emit_pv(g):
                j, o, u, d = pairs[g]
                if g == slot_first[j]:
                    ot_new = ot_ps.tile([P, QS_PER_CH, HS + 1], f32, tag="ot")
                    ots[j] = ot_new
                ot = ots[j]
                pt = pts.pop(g)
                first = g == slot_first[j]
                last = g == slot_last[j]
                for h in range(2):
                    for s in range(QS_PER_CH):
                        # start=True clears has_written for the WHOLE bank,
                        # so only the first write may set it; the other
                        # regions' first writes land on cleared bits and
                        # overwrite correctly.
                        nc.tensor.matmul(
                            ot[:, s, :],
                            lhsT=pt[:, h * CH + s * P : h * CH + (s + 1) * P],
                            rhs=v_all[:, o, u, d + 2 * h, :],
                            start=(first and h == 0 and s == 0),
                            stop=(last and h == 1 and s == QS_PER_CH - 1),
                        )
                if last:
                    finalize(j)

            def finalize(j):
                ot = ots.pop(j)
                fo = fin_pool.tile([P, QS_PER_CH, HS], f32, tag="fo")
                for s in range(QS_PER_CH):
                    rec = fin_pool.tile([P, 1], f32, tag="rec")
                    nc.vector.reciprocal(rec[:], ot[:, s, HS : HS + 1])
                    nc.vector.tensor_scalar_mul(fo[:, s, :], ot[:, s, 0:HS], rec[:])
                r0 = j * CH
                nc.sync.dma_start(
                    out_d[r0 : r0 + CH, :].rearrange("(s ci) m -> ci s m", ci=P),
                    fo[:],
                )

            for g in range(GT + LAG):
                if g < GT:
                    j, o, u, d = pairs[g]
                    # just-in-time: this pair's chunk and the slot's own
                    # diagonal chunk (source of Q^T) must be emitted
                    min_need = max(chunk_pos[(o, u)], chunk_pos[(0, j)])
                    if min_need > pumped[0]:
                        pump(min_need - pumped[0])
                    emit_scores(g)
                    # spread remaining projection emission toward the next
                    # slot's own chunk to keep the PE fed
                    nxt = chunk_pos[(0, j + 1)] if j + 1 < NSLOT else cum[-1]
                    span = slot_last[j] - slot_first[j] + 1
                    frac = (g - slot_first[j] + 1) / span
                    want = int(pumped[0] + (nxt - pumped[0]) * frac)
                    if want > pumped[0]:
                        pump(want - pumped[0])
                if g >= LAG:
                    emit_pv(g - LAG)

    nc.compile()
    return nc


_CACHE = {}


def _get_program():
    if "nc" not in _CACHE:
        _CACHE["nc"] = _build_program()
    return _CACHE["nc"]


def _host_inputs(x, Wk, Wq, Wv):
    import ml_dtypes

    bf16 = ml_dtypes.bfloat16
    x = np.asarray(x, dtype=np.float32)
    wqk = np.ascontiguousarray(
        np.concatenate([np.asarray(Wq), np.asarray(Wk)], axis=1).astype(bf16)
    )
    wv = np.ascontiguousarray(np.asarray(Wv, dtype=np.float32).astype(bf16))

    # x per core, tiled [ci, local chunk, co, t]: own u0..u3 then partner
    xTs = {}
    for b in range(B):
        xb = x[b].T.astype(bf16)  # [C, T]
        for p in range(2):
            chunks = [2 * u + p for u in range(NSLOT)] + [
                2 * u + (1 - p) for u in range(NSLOT)
            ]
            cols = np.concatenate([np.arange(c * CH, (c + 1) * CH) for c in chunks])
            xs = xb[:, cols].reshape(CSUB, P, NCH, CH)  # (co, ci, lc, t)
            xTs[(b, p)] = np.ascontiguousarray(xs.transpose(1, 2, 0, 3))

    # mask planes [128, o, d, 1024]
    mask2s = []
    rr = np.arange(P)[:, None]
    cc = np.arange(CH)[None, :]
    for p in range(2):
        m2 = np.empty((P, 2, 2, 2 * CH), dtype=np.float32)
        for d in range(2):
            for h in range(2):
                # multiplicative masks: o=0 diagonal staircase for key tile
                # d + 2h (dual-tile layout), visible iff c >= 128*(d+2h) + r
                m2[:, 0, d, h * CH : (h + 1) * CH] = np.where(
                    cc >= 128 * (d + 2 * h) + rr, 1.0, 0.0
                )
                # o=1: pad plane (p=0 zeroes its beyond-causal partner chunk)
                m2[:, 1, d, h * CH : (h + 1) * CH] = 1.0 if p == 1 else 0.0
        mask2s.append(m2.astype(bf16))

    in_maps = []
    for core in range(2 * B):
        b, p = core // 2, core % 2
        in_maps.append(
            {
                "xT": xTs[(b, p)],
                "wqk": wqk,
                "wv": wv,
                "mask2": mask2s[p],
            }
        )
    return in_maps


def _assemble(results):
    out = np.empty((B, T, HS), dtype=np.float32)
    for core in range(2 * B):
        b, p = core // 2, core % 2
        oc = results[core]["out"]
        for j in range(NSLOT):
            g = 2 * j + p
            out[b, g * CH : (g + 1) * CH, :] = oc[j * CH : (j + 1) * CH, :]
    return out


def run(x, Wk, Wq, Wv, trace=False):
    nc = _get_program()
    in_maps = _host_inputs(x, Wk, Wq, Wv)
    res = run_bass_kernel_spmd(nc, in_maps, list(range(2 * B)), trace=trace)
    return _assemble(res.results), res


def kernel(x, Wk, Wq, Wv):
    out, _ = run(x, Wk, Wq, Wv)
    return out


# revision 59
# speedup vs baseline: 1.1908x; 1.1908x over previous
"""Causal single-head attention on 8 Trainium2 NeuronCores.

Problem: x[4, 4096, 1024], Wq/Wk/Wv[1024, 64] ->
  out = softmax(causal(Q K^T / 8)) V   per batch, fp32.

Sharding: core i handles batch b = i//2 with query-chunk parity p = i%2
(512-wide query chunks; core p owns global chunks {p, 2+p, 4+p, 6+p}).
Both cores of a pair project the full x[b] (pair collectives measured too
slow here, ~40us each, to be worth deduplicating the projections).

The SPMD program is identical on all cores. Parity enters only through data:
  - the host-sliced x columns (own 4 chunks, then partner 4 chunks)
  - multiplicative 0/1 mask planes [128, o, d, 1024]: (o=own, u=j) is the
    diagonal staircase (parity-independent values); (o=partner, u=j) is
    all-zero on p=0 (pad work for SPMD balance) and all-ones on p=1.
K/V state is indexed [own/partner][u]; attention visits key chunks in
per-core order, which is legal because softmax and PV accumulation are
order-free over keys.

Pipeline structure: each score pair computes two 128-key tiles CONCURRENTLY
(dual-tile 64x128 row-tiled PE: T0 on partitions 0-63, T8 on 64-127) into one
2-bank PSUM tile [128, 1024]; exp is one scalar ACTIVATE per pair; the causal
mask is a multiplicative 0/1 bf16 vector op AFTER exp, off the scores->exp
chain; PV matmuls lag scores by LAG pairs so the in-order PE queue never
waits on the scalar engine; projection matmuls for later chunks interleave
into the attention loop as PE filler (keeps HAM at K=8/8 = 2.4 GHz).

On-device layout: scores are computed transposed (S^T[k, q] = K^T.T Q^T per
128x512 block). PV then uses the softmax'd P^T tile as the *stationary*
operand and V (augmented with a ones column accumulating the softmax
denominator) as the moving operand: out[q, h+1] = P^T.T @ V_aug lands
directly in natural [q, h] layout, so no final PE transpose is needed and
normalization reads the PSUM accumulator in place.

Operands are bf16 (enables Fast Weight Load on 128-col stationaries and
halves DMA); accumulation is fp32 in PSUM. Measured rel err ~4.4e-3.

Measured HW exec: ~80-82us on a rested device; ~96us when the device's
firmware power throttler (activity_1, 0.5 util cap) is hot from
back-to-back runs. The compute span is roofline-bound: tensor and scalar
engines both ~90% busy at the warm 2.4 GHz clock, finishing together.
"""

import numpy as np

import concourse.bacc as bacc
import concourse.mybir as mybir
import concourse.tile as tile
from concourse.bass_utils import run_bass_kernel_spmd

# Problem dims
B, T, C, HS = 4, 4096, 1024, 64
P = 128           # partitions
CH = 512          # query-chunk width
NCH = T // CH     # 8 global chunks
NSLOT = NCH // 2  # 4 local query slots / own chunks per core
CSUB = C // P     # 8 contraction subtiles
KT_PER_CH = CH // P   # 4 key tiles (128) per chunk
QS_PER_CH = CH // P   # 4 query subtiles (128) per chunk
NEG = -1.0e9
LAG = 2           # pair-pipeline depth: PV for pair g emitted at step g+LAG

IN_DT = mybir.dt.bfloat16  # matmul operand storage dtype


def _build_program():
    nc = bacc.Bacc("TRN2")
    f32 = mybir.dt.float32
    EXP = mybir.ActivationFunctionType.Exp

    # host-pretiled x: [ci, local chunk, co, t] with own u0..u3 then partner
    # u0..u3 on the chunk axis; each chunk half-DMA reads 4KB/partition lines
    xT = nc.dram_tensor(
        "xT", [P, NCH, CSUB, CH], IN_DT, kind="ExternalInput"
    ).ap()
    wqk = nc.dram_tensor("wqk", [C, 2 * HS], IN_DT, kind="ExternalInput").ap()
    wv = nc.dram_tensor("wv", [C, HS], IN_DT, kind="ExternalInput").ap()
    # causal mask planes: [128, o, d, 1024]
    mask2_d = nc.dram_tensor(
        "mask2", [P, 2, 2, 2 * CH], IN_DT, kind="ExternalInput"
    ).ap()
    out_d = nc.dram_tensor("out", [NSLOT * CH, HS], f32, kind="ExternalOutput").ap()

    wqk_r = wqk.rearrange("(co ci) m -> ci co m", ci=P)    # [128, 8, 128]
    wv_r = wv.rearrange("(co ci) m -> ci co m", ci=P)      # [128, 8, 64]

    with tile.TileContext(nc) as tc:
        with (
            tc.tile_pool(name="const", bufs=1) as const_pool,
            tc.tile_pool(name="persist", bufs=1) as persist,
            tc.tile_pool(name="xin", bufs=8) as xpool,
            tc.tile_pool(name="pt", bufs=6) as pt_pool,
            tc.tile_pool(name="fin", bufs=2) as fin_pool,
            tc.tile_pool(name="proj_ps", bufs=2, space="PSUM") as proj_ps,
            tc.tile_pool(name="st_ps", bufs=2, space="PSUM") as st_ps,
            tc.tile_pool(name="ot_ps", bufs=2, space="PSUM") as ot_ps,
        ):
            # ---- constants (gpsimd DMA queue; sync queue is for x) ----
            wqk_sb = const_pool.tile([P, CSUB, 2 * HS], IN_DT)
            wv_sb = const_pool.tile([P, CSUB, HS], IN_DT)
            mask2_sb = const_pool.tile([P, 2, 2, 2 * CH], IN_DT)
            nc.gpsimd.dma_start(wqk_sb[:], wqk_r)

            # K^T packed for dual-tile scores: keys 0-255 of a chunk in
            # partitions 0-63 (PE tile T0), keys 256-511 in partitions
            # 64-127 (tile T8) -> two score matmuls run CONCURRENTLY in the
            # row-tiled 64x128 PE configuration.
            kt_all = persist.tile([P, 2, NSLOT, CH // 2], IN_DT)
            v_all = persist.tile([P, 2, NSLOT, KT_PER_CH, HS + 1], IN_DT)
            # Q^T duplicated into both partition halves (T0 and T8 streams)
            qt_slot = persist.tile([P, NSLOT, CH], IN_DT)
            # 0x3F80 = 1.0 in bf16
            nc.vector.memset(
                v_all[:, :, :, :, HS : HS + 1].bitcast(mybir.dt.uint16), 0x3F80
            )



            # ---- local projection of one chunk --------------------------
            def proj_chunk_steps(lc, o, u):
                xc = xpool.tile([P, CSUB, CH], IN_DT, tag="xc")
                nc.sync.dma_start(xc[:, 0:4, :], xT[:, lc, 0:4, :])
                yield
                nc.sync.dma_start(xc[:, 4:8, :], xT[:, lc, 4:8, :])
                if lc == 0:
                    # deferred consts: wv needed at ~18us, mask at ~22us --
                    # keep them off the DMA engines while xc(0) streams
                    nc.gpsimd.dma_start(wv_sb[:], wv_r)
                    nc.gpsimd.dma_start(mask2_sb[:], mask2_d)
                yield

                # Q^T (rows 0:64) and K^T (rows 64:128), stacked projection
                qk_ps = proj_ps.tile([P, CH], f32, tag="proj")
                for cs in range(CSUB):
                    nc.tensor.matmul(
                        qk_ps[:],
                        lhsT=wqk_sb[:, cs, :],
                        rhs=xc[:, cs, :],
                        start=(cs == 0),
                        stop=(cs == CSUB - 1),
                    )
                    yield
                if o == 0:
                    nc.vector.tensor_copy(qt_slot[0:HS, u, :], qk_ps[0:HS, :])
                    nc.vector.tensor_copy(qt_slot[HS:P, u, :], qk_ps[0:HS, :])
                    yield
                nc.vector.tensor_copy(
                    kt_all[0:HS, o, u, :], qk_ps[HS:P, 0 : CH // 2]
                )
                nc.vector.tensor_copy(
                    kt_all[HS:P, o, u, :], qk_ps[HS:P, CH // 2 : CH]
                )
                yield

                # V natural ([t, h]) via x^T blocks as stationary operand
                v_ps = proj_ps.tile([P, KT_PER_CH, HS], f32, tag="proj")
                for tt in range(KT_PER_CH):
                    for cs in range(CSUB):
                        nc.tensor.matmul(
                            v_ps[:, tt, :],
                            lhsT=xc[:, cs, tt * P : (tt + 1) * P],
                            rhs=wv_sb[:, cs, :],
                            start=(cs == 0),
                            stop=(cs == CSUB - 1),
                        )
                    yield
                nc.vector.tensor_copy(v_all[:, o, u, :, 0:HS], v_ps[:])
                yield

            # emission order: own chunk j carries slot j's Q^T, so own
            # chunks come first within each causal stage; the partner diag
            # chunk is only needed by the last two pairs of its slot
            plan = [
                (0, 0, 0),   # own u0
                (4, 1, 0),   # partner u0
                (1, 0, 1),   # own u1
                (5, 1, 1),   # partner u1
                (2, 0, 2),   # own u2
                (6, 1, 2),   # partner u2
                (3, 0, 3),   # own u3
                (7, 1, 3),   # partner u3
            ]

            def steps_of(spec):
                lc, o, u = spec
                return 16 + (1 if o == 0 else 0)

            def chained(gens):
                for g in gens:
                    yield from g

            projgen = chained(proj_chunk_steps(*spec) for spec in plan)
            cum = np.cumsum([steps_of(s) for s in plan]).tolist()
            # emission watermark per projected chunk (o, u)
            chunk_pos = {
                (o, u): cum[i] for i, (lc, o, u) in enumerate(plan)
            }
            pumped = [0]

            def pump(n):
                for _ in range(n):
                    if next(projgen, "done") == "done":
                        return
                    pumped[0] += 1

            pump(chunk_pos[(0, 0)])

            # ---- attention: one flat software pipeline across all slots --
            # global pair list: (j, o, u, d); within a slot the diagonal/pad
            # chunks (u == j) come last
            pairs = []
            slot_first = []
            slot_last = []
            for j in range(NSLOT):
                slot_first.append(len(pairs))
                pairs += [
                    (j, o, u, d)
                    for u in range(j + 1)
                    for o in range(2)
                    for d in range(2)
                ]
                slot_last.append(len(pairs) - 1)
            GT = len(pairs)

            # natural-layout PV accumulator: [q, qs, h+1], one PSUM bank
            ots = {}
            pts = {}

            def emit_scores(g):
                j, o, u, d = pairs[g]
                st = st_ps.tile([P, 2 * CH], f32, tag="st")
                # st half h holds key tile d + 2h: h=0 on PE tile T0
                # (partitions 0-63), h=1 on T8 (64-127), concurrently
                for h in range(2):
                    r0 = h * HS
                    nc.tensor.matmul(
                        st[:, h * CH : (h + 1) * CH],
                        lhsT=kt_all[r0 : r0 + HS, o, u, d * P : (d + 1) * P],
                        rhs=qt_slot[r0 : r0 + HS, j, :],
                        start=True,
                        stop=True,
                    )
                pt = pt_pool.tile([P, 2 * CH], IN_DT, tag="pt")
                nc.scalar.activation(pt[:], st[:], EXP, scale=float(HS) ** -0.5)
                if u == j:
                    # multiplicative 0/1 causal mask AFTER exp: off the
                    # scores->exp critical chain (PV's LAG absorbs it) and
                    # bf16 runs at 2x DVE rate vs the old f32 PSUM add
                    nc.vector.tensor_mul(pt[:], pt[:], mask2_sb[:, o, d, :])
                pts[g] = pt

            def emit_pv(g):
                j, o, u, d = pairs[g]
                if g == slot_first[j]:
                    ot_new = ot_ps.tile([P, QS_PER_CH, HS + 1], f32, tag="ot")
                    ots[j] = ot_new
                ot = ots[j]
                pt = pts.pop(g)
                first = g == slot_first[j]
                last = g == slot_last[j]
                for h in range(2):
                    for s in range(QS_PER_CH):
                        # start=True clears has_written for the WHOLE bank,
                        # so only the first write may set it; the other
                        # regions' first writes land on cleared bits and
                        # overwrite correctly.
                        nc.tensor.matmul(
                            ot[:, s, :],
                            lhsT=pt[:, h * CH + s * P : h * CH + (s + 1) * P],
                            rhs=v_all[:, o, u, d + 2 * h, :],
                            start=(first and h == 0 and s == 0),
                            stop=(last and h == 1 and s == QS_PER_CH - 1),
                        )
                if last:
                    finalize(j)

            def finalize(j):
                ot = ots.pop(j)
                fo = fin_pool.tile([P, QS_PER_CH, HS], f32, tag="fo")
                for s in range(QS_PER_CH):
                    rec = fin_pool.tile([P, 1], f32, tag="rec")
                    nc.vector.reciprocal(rec[:], ot[:, s, HS : HS + 1])
                    nc.vector.tensor_scalar_mul(fo[:, s, :], ot[:, s, 0:HS], rec[:])
                r0 = j * CH
                nc.sync.dma_start(
                    out_d[r0 : r0 + CH, :].rearrange("(s ci) m -> ci s m", ci=P),
                    fo[:],
                )

            for g in range(GT + LAG):
                if g < GT:
                    j, o, u, d = pairs[g]
                    # just-in-time: this pair's chunk and the slot's own
                    # diagonal chunk (source of Q^T) must be emitted
                    min_need = max(chunk_pos[(o, u)], chunk_pos[(0, j)])
                    if min_need > pumped[0]:
                        pump(min_need - pumped[0])
                    emit_scores(g)
                    # spread remaining projection emission toward the next
                    # slot's own chunk to keep the PE fed
                    nxt = chunk_pos[(0, j + 1)] if j + 1 < NSLOT else cum[-1]
                    span = slot_last[j] - slot_first[j] + 1
                    frac = (g - slot_first[j] + 1) / span
                    want = int(pumped[0] + (nxt - pumped[0]) * frac)
                    if want > pumped[0]:
                        pump(want - pumped[0])
                if g >= LAG:
                    emit_pv(g - LAG)

    nc.compile()
    return nc


_CACHE = {}


def _get_program():
    if "nc" not in _CACHE:
        _CACHE["nc"] = _build_program()
    return _CACHE["nc"]


def _host_inputs(x, Wk, Wq, Wv):
    import ml_dtypes

    bf16 = ml_dtypes.bfloat16
    x = np.asarray(x, dtype=np.float32)
    wqk = np.ascontiguousarray(
        np.concatenate([np.asarray(Wq), np.asarray(Wk)], axis=1).astype(bf16)
    )
    wv = np.ascontiguousarray(np.asarray(Wv, dtype=np.float32).astype(bf16))

    # x per core, tiled [ci, local chunk, co, t]: own u0..u3 then partner
    xTs = {}
    for b in range(B):
        xb = x[b].T.astype(bf16)  # [C, T]
        for p in range(2):
            chunks = [2 * u + p for u in range(NSLOT)] + [
                2 * u + (1 - p) for u in range(NSLOT)
            ]
            cols = np.concatenate([np.arange(c * CH, (c + 1) * CH) for c in chunks])
            xs = xb[:, cols].reshape(CSUB, P, NCH, CH)  # (co, ci, lc, t)
            xTs[(b, p)] = np.ascontiguousarray(xs.transpose(1, 2, 0, 3))

    # mask planes [128, o, d, 1024]
    mask2s = []
    rr = np.arange(P)[:, None]
    cc = np.arange(CH)[None, :]
    for p in range(2):
        m2 = np.empty((P, 2, 2, 2 * CH), dtype=np.float32)
        for d in range(2):
            for h in range(2):
                # multiplicative masks: o=0 diagonal staircase for key tile
                # d + 2h (dual-tile layout), visible iff c >= 128*(d+2h) + r
                m2[:, 0, d, h * CH : (h + 1) * CH] = np.where(
                    cc >= 128 * (d + 2 * h) + rr, 1.0, 0.0
                )
                # o=1: pad plane (p=0 zeroes its beyond-causal partner chunk)
                m2[:, 1, d, h * CH : (h + 1) * CH] = 1.0 if p == 1 else 0.0
        mask2s.append(m2.astype(bf16))

    in_maps = []
    for core in range(2 * B):
        b, p = core // 2, core % 2
        in_maps.append(
            {
                "xT": xTs[(b, p)],
                "wqk": wqk,
                "wv": wv,
                "mask2": mask2s[p],
            }
        )
    return in_maps


def _assemble(results):
    out = np.empty((B, T, HS), dtype=np.float32)
    for core in range(2 * B):
        b, p = core // 2, core % 2
        oc = results[core]["out"]
        for j in range(NSLOT):
            g = 2 * j + p
            out[b, g * CH : (g + 1) * CH, :] = oc[j * CH : (j + 1) * CH, :]
    return out


def run(x, Wk, Wq, Wv, trace=False):
    nc = _get_program()
    in_maps = _host_inputs(x, Wk, Wq, Wv)
    res = run_bass_kernel_spmd(nc, in_maps, list(range(2 * B)), trace=trace)
    return _assemble(res.results), res


def kernel(x, Wk, Wq, Wv):
    out, _ = run(x, Wk, Wq, Wv)
    return out
